# revision 50
# baseline (speedup 1.0000x reference)
"""Trainium2 Bass kernel: MechanicsPINN residual (MLP field + biharmonic stencil).

Math (reference): f = MLP(x_coloc) -> [B, H*W]; residual = L(L(f)) + L(f) + f - P
where L is the 5-point reflect-padded Laplacian (EI = KC = GC = 1, dx = dy = 1).

Sharding: tensor-parallel over the 65536 output pixels = 256 image rows.
Core c owns rows [32c, 32c+32). Each core recomputes the tiny MLP, then
computes f for its rows plus a 2-row halo on each side (mirror boundary rows
are folded in on the host by remapping which W4 columns each core streams,
so the device stencil needs no y-boundary cases and no cross-core comms).

Device layout: batch (64) on partitions; each core's 32 rows are split into
two 16-row halves stacked on the partition axis (partitions 0-63 = batch for
half A, 64-127 = batch for half B) via PE column-tiling, so DVE stencil
passes and the big matmul both use all 128 partitions. W4/W2/W3/P are
host-cast to bf16 (halves the HBM traffic that bounds this kernel); stencil
intermediates are bf16 with the final combine accumulated to fp32.
"""

import numpy as np
import ml_dtypes

import concourse.bass as bass
import concourse.tile as tile
from concourse import bacc, mybir
from concourse.bass_utils import run_bass_kernel_spmd

F32 = mybir.dt.float32
BF16 = mybir.dt.bfloat16
BF16_NP = ml_dtypes.bfloat16

B = 64          # batch (collocation samples)
H = 256         # image rows
W = 256         # image cols
NCORES = 8
OWN = 32        # image rows owned per core
HR = 16         # rows per half-slab
FR = 20         # f rows held per half (HR + 2 halo each side)
LR = 18         # laplacian rows per half (HR + 1 each side)
PAIRS = 10      # B-half 512-col chunks (2 rows each), streamed in order
NA = 8          # A-half chunks: rows 0..15; rows 16..19 are copied from the
                # B half (B rows 0..3 are the same global rows 14..17)
KT = 8          # k tiles of the 1024-dim contraction

_PROGRAM_CACHE = {}


def _mirror(j):
    # jnp.pad mode='reflect' (no edge repeat): p[-1] = f[1], p[H] = f[H-2]
    if j < 0:
        return -j
    if j > H - 1:
        return 2 * (H - 1) - j
    return j


def _build_program():
    nc = bacc.Bacc("TRN2", target_bir_lowering=False, debug=False)

    xT = nc.declare_dram_parameter("xT", [2, B], F32, isOutput=False)
    W1 = nc.declare_dram_parameter("W1", [2, 256], F32, isOutput=False)
    W2 = nc.declare_dram_parameter("W2", [128, 2, 512], BF16, isOutput=False)
    W3 = nc.declare_dram_parameter("W3", [128, 4, 1024], BF16, isOutput=False)
    bias = nc.declare_dram_parameter("bias", [128, 14], F32, isOutput=False)
    W4A = nc.declare_dram_parameter("W4A", [NA, 128, KT, 512], BF16, isOutput=False)
    W4B = nc.declare_dram_parameter("W4B", [PAIRS, 128, KT, 512], BF16, isOutput=False)
    b4s = nc.declare_dram_parameter("b4s", [PAIRS, 1, 1024], BF16, isOutput=False)
    eye = nc.declare_dram_parameter("eye", [128, 64], BF16, isOutput=False)
    Ps = nc.declare_dram_parameter("Ps", [128, HR * W], BF16, isOutput=False)
    out = nc.declare_dram_parameter("out", [128, HR * W], F32, isOutput=True)

    Relu = mybir.ActivationFunctionType.Relu
    MUL = mybir.AluOpType.mult
    ADD = mybir.AluOpType.add

    with tile.TileContext(nc) as tc:
        with (
            tc.tile_pool(name="singles", bufs=1) as singles,
            tc.tile_pool(name="wpool", bufs=6) as wpool,
            tc.tile_pool(name="bpool", bufs=3) as bpool,
            tc.tile_pool(name="spool", bufs=2) as spool,
            tc.tile_pool(name="tpool", bufs=2) as tpool,
            tc.tile_pool(name="rpool", bufs=2) as rpool,
        ):
            dma = nc.sync.dma_start

            def fetch_pair(i):
                wa = None
                if i < NA:
                    wa = wpool.tile([128, KT, 512], BF16, tag="wa")
                    dma(out=wa[:, :, :], in_=W4A[i])
                wb = wpool.tile([128, KT, 512], BF16, tag="wb")
                dma(out=wb[:, :, :], in_=W4B[i])
                bt = bpool.tile([1, 1024], BF16, tag="bt")
                dma(out=bt[:, :], in_=b4s[i])
                return wa, wb, bt

            xT_sb = singles.tile([2, B], F32)
            W1_sb = singles.tile([2, 256], F32)
            W2_sb = singles.tile([128, 2, 512], BF16)
            W3_sb = singles.tile([128, 4, 1024], BF16)
            bias_sb = singles.tile([128, 14], F32)
            h1_sb = singles.tile([128, 2, B], BF16)
            h2_sb = singles.tile([128, 4, B], BF16)
            h3_sb = singles.tile([128, KT, B], BF16)
            ones = singles.tile([1, B], BF16)
            eye_sb = singles.tile([128, 64], BF16)
            Ft = singles.tile([128, FR * W], BF16)
            Lf = singles.tile([128, LR * W], BF16)
            Ps_sb = singles.tile([128, HR * W], BF16)

            dma(out=xT_sb[:, :], in_=xT[:, :])
            dma(out=W1_sb[:, :], in_=W1[:, :])
            dma(out=bias_sb[:, :], in_=bias[:, :])
            dma(out=W2_sb[:, :, :], in_=W2[:, :, :])
            dma(out=W3_sb[:, :, :], in_=W3[:, :, :])
            dma(out=eye_sb[:, :], in_=eye[:, :])
            nc.vector.memset(ones, 1.0)

            # ---- MLP (transposed activations: h_T[feat, batch]) ----
            with tc.tile_pool(name="mlp_psum", bufs=2, space="PSUM") as mp:
                for m in range(2):
                    ps = mp.tile([128, B], F32)
                    nc.tensor.matmul(
                        ps, W1_sb[:, m * 128 : (m + 1) * 128], xT_sb[:, :],
                        start=True, stop=True,
                    )
                    nc.scalar.activation(
                        h1_sb[:, m, :], ps, Relu, bias=bias_sb[:, m : m + 1], scale=1.0
                    )
                for m in range(4):
                    ps = mp.tile([128, B], F32)
                    for k in range(2):
                        nc.tensor.matmul(
                            ps, W2_sb[:, k, m * 128 : (m + 1) * 128], h1_sb[:, k, :],
                            start=(k == 0), stop=(k == 1),
                        )
                    nc.scalar.activation(
                        h2_sb[:, m, :], ps, Relu, bias=bias_sb[:, 2 + m : 3 + m], scale=1.0
                    )
                for m in range(8):
                    ps = mp.tile([128, B], F32)
                    for k in range(4):
                        nc.tensor.matmul(
                            ps, W3_sb[:, k, m * 128 : (m + 1) * 128], h2_sb[:, k, :],
                            start=(k == 0), stop=(k == 3),
                        )
                    nc.scalar.activation(
                        h3_sb[:, m, :], ps, Relu, bias=bias_sb[:, 6 + m : 7 + m], scale=1.0
                    )

            # ---- main matmul: F[p, 512-col chunks], half A -> partitions 0-63,
            # half B -> partitions 64-127 (PE column groups run concurrently) ----
            with (
                tc.tile_pool(name="ppool", bufs=2, space="PSUM") as ppool,
                tc.tile_pool(name="prpool", bufs=1, space="PSUM") as prpool,
            ):
                for i in range(PAIRS):
                    wa, wb, bt = fetch_pair(i)
                    has_a = wa is not None
                    # half A accumulates in bank 0 (partitions 0-63), half B in
                    # bank 1 (partitions 64-127): separate psum zero regions,
                    # concurrent PE column groups.
                    ps = ppool.tile([128, 1024], F32)
                    for k in range(KT):
                        if has_a:
                            nc.tensor.matmul(
                                ps[0:64, 0:512], h3_sb[:, k, :], wa[:, k, :],
                                start=(k == 0), stop=False, tile_position=(0, 0),
                            )
                        nc.tensor.matmul(
                            ps[64:128, 512:1024], h3_sb[:, k, :], wb[:, k, :],
                            start=(k == 0), stop=False, tile_position=(0, 64),
                        )
                    if has_a:
                        nc.tensor.matmul(
                            ps[0:64, 0:512], ones[:, :], bt[:, 0:512],
                            start=False, stop=True, tile_position=(0, 0),
                        )
                        nc.scalar.copy(Ft[0:64, i * 512 : (i + 1) * 512], ps[0:64, 0:512])
                    nc.tensor.matmul(
                        ps[64:128, 512:1024], ones[:, :], bt[:, 512:1024],
                        start=False, stop=True, tile_position=(0, 64),
                    )
                    nc.scalar.copy(
                        Ft[64:128, i * 512 : (i + 1) * 512], ps[64:128, 512:1024]
                    )
                    if i == 1:
                        # A-half top rows 16..19 = B-half rows 0..3 (both are
                        # global rows 14..17): SBUF->SBUF partition copy
                        # instead of re-streaming 2.1MB of W4.
                        dma(out=Ft[0:64, 4096:5120], in_=Ft[64:128, 0:1024])

                # ---- stencils, slab-pipelined & interleaved so DVE overlaps
                # with the matmul/DMA stream ----
                Fv = Ft.rearrange("p (r x) -> p r x", x=W)
                Lfv = Lf.rearrange("p (r x) -> p r x", x=W)
                STT = nc.vector.scalar_tensor_tensor

                def lf_slab(j):
                    # Lf rows 3j..3j+2 (center = F row+1)
                    n = 3 * W
                    r0 = 3 * j
                    cb = (r0 + 1) * W
                    s1 = spool.tile([128, n], BF16, tag="s1")
                    s2 = spool.tile([128, n], BF16, tag="s2")
                    nc.vector.tensor_add(s1, Ft[:, cb - 1 : cb - 1 + n], Ft[:, cb + 1 : cb + 1 + n])
                    s1v = s1.rearrange("p (r x) -> p r x", x=W)
                    nc.scalar.mul(s1v[:, :, 0:1], Fv[:, r0 + 1 : r0 + 4, 1:2], 2.0)
                    nc.scalar.mul(s1v[:, :, W - 1 : W], Fv[:, r0 + 1 : r0 + 4, W - 2 : W - 1], 2.0)
                    nc.vector.tensor_add(s2, Ft[:, cb - W : cb - W + n], Ft[:, cb + W : cb + W + n])
                    STT(out=s1, in0=Ft[:, cb : cb + n], scalar=-4.0, in1=s1, op0=MUL, op1=ADD)
                    nc.vector.tensor_add(Lf[:, r0 * W : r0 * W + n], s1, s2)

                def r_slab(m):
                    # residual rows 4m..4m+3 (centers: Lf row+1, F row+2)
                    n = 4 * W
                    r0 = 4 * m
                    lb = (r0 + 1) * W
                    fb = (r0 + 2) * W
                    ob = r0 * W
                    # scalar-engine DMA queue: lands promptly instead of
                    # waiting behind the W4 stream in the sync queue
                    nc.scalar.dma_start(out=Ps_sb[:, ob : ob + n], in_=Ps[:, ob : ob + n])
                    t1 = tpool.tile([128, n], BF16, tag="t1")
                    t2 = tpool.tile([128, n], BF16, tag="t2")
                    t5 = tpool.tile([128, n], BF16, tag="t5")
                    rt = rpool.tile([128, n], F32, tag="rt")
                    nc.vector.tensor_add(t1, Lf[:, lb - 1 : lb - 1 + n], Lf[:, lb + 1 : lb + 1 + n])
                    t1v = t1.rearrange("p (r x) -> p r x", x=W)
                    nc.scalar.mul(t1v[:, :, 0:1], Lfv[:, r0 + 1 : r0 + 5, 1:2], 2.0)
                    nc.scalar.mul(t1v[:, :, W - 1 : W], Lfv[:, r0 + 1 : r0 + 5, W - 2 : W - 1], 2.0)
                    nc.vector.tensor_add(t2, Lf[:, lb - W : lb - W + n], Lf[:, lb + W : lb + W + n])
                    STT(out=t1, in0=Lf[:, lb : lb + n], scalar=-3.0, in1=t1, op0=MUL, op1=ADD)
                    nc.vector.tensor_sub(t5, Ft[:, fb : fb + n], Ps_sb[:, ob : ob + n])
                    # final 3-way add on the PE via identity matmuls: psum
                    # accumulates t1+t2+t5 in fp32. A half -> banks 0-1,
                    # B half -> banks 2-3 (no shared accumulation groups).
                    pr = prpool.tile([128, 2048], F32)
                    for c in (0, 1):
                        sl = slice(c * 512, (c + 1) * 512)
                        for j, src in enumerate((t1, t2, t5)):
                            nc.tensor.matmul(
                                pr[0:64, c * 512 : (c + 1) * 512],
                                eye_sb[0:64, :], src[0:64, sl],
                                start=(j == 0), stop=(j == 2), tile_position=(0, 0),
                            )
                        for j, src in enumerate((t1, t2, t5)):
                            nc.tensor.matmul(
                                pr[64:128, 1024 + c * 512 : 1024 + (c + 1) * 512],
                                eye_sb[64:128, :], src[64:128, sl],
                                start=(j == 0), stop=(j == 2), tile_position=(64, 64),
                            )
                    nc.any.tensor_copy(rt[0:64, :], pr[0:64, 0:1024])
                    nc.any.tensor_copy(rt[64:128, :], pr[64:128, 1024:2048])
                    dma(out=out[:, ob : ob + n], in_=rt[:, :])

                # R slab m reads Lf rows 4m..4m+5, so it must follow Lf slab
                # ceil((4m+5+1)/3)-1.
                for step in ("L0", "L1", "R0", "L2", "L3", "R1", "L4", "R2", "L5", "R3"):
                    kind, idx = step[0], int(step[1])
                    if kind == "L":
                        lf_slab(idx)
                    else:
                        r_slab(idx)

    nc.compile()
    return nc


def _ext_rows(c):
    """40 mirrored global row indices: 20 for half A, 20 for half B."""
    y0 = c * OWN
    rows_a = [_mirror(y0 - 2 + j) for j in range(FR)]
    rows_b = [_mirror(y0 + HR - 2 + j) for j in range(FR)]
    return rows_a + rows_b


def _prep_shared(inputs):
    f32 = np.float32
    shared = {
        "xT": np.ascontiguousarray(inputs["x_coloc"].T, dtype=f32),
        "W1": np.ascontiguousarray(inputs["W1"], dtype=f32),
        "W2": np.ascontiguousarray(
            np.asarray(inputs["W2"], dtype=f32).reshape(2, 128, 512).transpose(1, 0, 2).astype(BF16_NP)
        ),
        "W3": np.ascontiguousarray(
            np.asarray(inputs["W3"], dtype=f32).reshape(4, 128, 1024).transpose(1, 0, 2).astype(BF16_NP)
        ),
        "bias": np.ascontiguousarray(
            np.concatenate(
                [
                    np.asarray(inputs["b1"], dtype=f32).reshape(2, 128).T,
                    np.asarray(inputs["b2"], dtype=f32).reshape(4, 128).T,
                    np.asarray(inputs["b3"], dtype=f32).reshape(8, 128).T,
                ],
                axis=1,
            )
        ),
        "eye": np.ascontiguousarray(
            np.tile(np.eye(64, dtype=np.float32), (2, 1)).astype(BF16_NP)
        ),
    }
    return shared


def _prep_core(c, W4, b4, P):
    y0 = c * OWN
    # A half: F rows -2..13 streamed (local rows 0..15); local rows 16..19 are
    # copied on-device from the B half. B half: F rows 14..33 fully streamed.
    rows_a = [_mirror(y0 - 2 + j) for j in range(2 * NA)]
    rows_b = [_mirror(y0 + HR - 2 + j) for j in range(FR)]

    W4r = W4.reshape(1024, H, W)

    def chunks(rows):
        G = W4r[:, rows, :].reshape(KT, 128, len(rows) // 2, 512)  # [k,p,chunk,x]
        G = G.transpose(2, 1, 0, 3)                                # [chunk,p,k,x]
        return np.ascontiguousarray(G.astype(BF16_NP))

    W4A_arr = chunks(rows_a)
    W4B_arr = chunks(rows_b)

    gb_a = b4.reshape(H, W)[rows_a].reshape(NA, 512)
    gb_b = b4.reshape(H, W)[rows_b].reshape(PAIRS, 512)
    b4s_arr = np.zeros((PAIRS, 1, 1024), dtype=BF16_NP)
    b4s_arr[:NA, 0, :512] = gb_a.astype(BF16_NP)
    b4s_arr[:, 0, 512:] = gb_b.astype(BF16_NP)

    Pr = P.reshape(B, H, W)
    Ps = np.concatenate(
        [
            Pr[:, y0 : y0 + HR, :].reshape(B, HR * W),
            Pr[:, y0 + HR : y0 + OWN, :].reshape(B, HR * W),
        ],
        axis=0,
    ).astype(BF16_NP)
    return {
        "W4A": W4A_arr, "W4B": W4B_arr, "b4s": b4s_arr,
        "Ps": np.ascontiguousarray(Ps),
    }


def make_in_maps(inputs):
    shared = _prep_shared(inputs)
    W4 = np.asarray(inputs["W4"], dtype=np.float32)
    b4 = np.asarray(inputs["b4"], dtype=np.float32)
    P = np.asarray(inputs["P"], dtype=np.float32)
    in_maps = []
    for c in range(NCORES):
        m = dict(shared)
        m.update(_prep_core(c, W4, b4, P))
        in_maps.append(m)
    return in_maps


def assemble_output(results):
    outf = np.empty((B, H, W), dtype=np.float32)
    for c in range(NCORES):
        oc = np.asarray(results[c]["out"])
        y0 = c * OWN
        outf[:, y0 : y0 + HR, :] = oc[:64].reshape(B, HR, W)
        outf[:, y0 + HR : y0 + OWN, :] = oc[64:].reshape(B, HR, W)
    return outf.reshape(B, H * W)


def get_program():
    if "nc" not in _PROGRAM_CACHE:
        _PROGRAM_CACHE["nc"] = _build_program()
    return _PROGRAM_CACHE["nc"]


def kernel(**inputs):
    nc = get_program()
    in_maps = make_in_maps(inputs)
    res = run_bass_kernel_spmd(nc, in_maps, list(range(NCORES)))
    return assemble_output(res.results)


# revision 52
# speedup vs baseline: 1.0240x; 1.0240x over previous
"""Trainium2 Bass kernel: MechanicsPINN residual (MLP field + biharmonic stencil).

Math (reference): f = MLP(x_coloc) -> [B, H*W]; residual = L(L(f)) + L(f) + f - P
where L is the 5-point reflect-padded Laplacian (EI = KC = GC = 1, dx = dy = 1).

Sharding: tensor-parallel over the 65536 output pixels = 256 image rows.
Core c owns rows [32c, 32c+32). Each core recomputes the tiny MLP, then
computes f for its rows plus a 2-row halo on each side (mirror boundary rows
are folded in on the host by remapping which W4 columns each core streams,
so the device stencil needs no y-boundary cases and no cross-core comms).

Device layout: batch (64) on partitions; each core's 32 rows are split into
two 16-row halves stacked on the partition axis (partitions 0-63 = batch for
half A, 64-127 = batch for half B) via PE column-tiling, so DVE stencil
passes and the big matmul both use all 128 partitions. W4/W2/W3/P are
host-cast to bf16 (halves the HBM traffic that bounds this kernel); stencil
intermediates are bf16 with the final combine accumulated to fp32.
"""

import numpy as np
import ml_dtypes

import concourse.bass as bass
import concourse.tile as tile
from concourse import bacc, mybir
from concourse.bass_utils import run_bass_kernel_spmd

F32 = mybir.dt.float32
BF16 = mybir.dt.bfloat16
BF16_NP = ml_dtypes.bfloat16

B = 64          # batch (collocation samples)
H = 256         # image rows
W = 256         # image cols
NCORES = 8
OWN = 32        # image rows owned per core
HR = 16         # rows per half-slab
FR = 20         # f rows held per half (HR + 2 halo each side)
LR = 18         # laplacian rows per half (HR + 1 each side)
PAIRS = 10      # B-half 512-col chunks (2 rows each), streamed in order
NA = 8          # A-half chunks: rows 0..15; rows 16..19 are copied from the
                # B half (B rows 0..3 are the same global rows 14..17)
KT = 8          # k tiles of the 1024-dim contraction

_PROGRAM_CACHE = {}


def _mirror(j):
    # jnp.pad mode='reflect' (no edge repeat): p[-1] = f[1], p[H] = f[H-2]
    if j < 0:
        return -j
    if j > H - 1:
        return 2 * (H - 1) - j
    return j


def _build_program():
    nc = bacc.Bacc("TRN2", target_bir_lowering=False, debug=False)

    xT = nc.declare_dram_parameter("xT", [2, B], F32, isOutput=False)
    W1 = nc.declare_dram_parameter("W1", [2, 256], F32, isOutput=False)
    W2 = nc.declare_dram_parameter("W2", [128, 2, 512], BF16, isOutput=False)
    W3 = nc.declare_dram_parameter("W3", [128, 4, 1024], BF16, isOutput=False)
    bias = nc.declare_dram_parameter("bias", [128, 14], F32, isOutput=False)
    W4A = nc.declare_dram_parameter("W4A", [NA, 128, KT, 512], BF16, isOutput=False)
    W4B = nc.declare_dram_parameter("W4B", [PAIRS, 128, KT, 512], BF16, isOutput=False)
    b4s = nc.declare_dram_parameter("b4s", [PAIRS, 1, 1024], BF16, isOutput=False)
    eye = nc.declare_dram_parameter("eye", [128, 64], BF16, isOutput=False)
    Ps = nc.declare_dram_parameter("Ps", [128, HR * W], BF16, isOutput=False)
    out = nc.declare_dram_parameter("out", [128, HR * W], F32, isOutput=True)

    Relu = mybir.ActivationFunctionType.Relu
    MUL = mybir.AluOpType.mult
    ADD = mybir.AluOpType.add

    with tile.TileContext(nc) as tc:
        with (
            tc.tile_pool(name="singles", bufs=1) as singles,
            tc.tile_pool(name="wpool", bufs=6) as wpool,
            tc.tile_pool(name="bpool", bufs=3) as bpool,
            tc.tile_pool(name="spool", bufs=2) as spool,
            tc.tile_pool(name="tpool", bufs=2) as tpool,
            tc.tile_pool(name="rpool", bufs=2) as rpool,
        ):
            dma = nc.sync.dma_start

            def fetch_pair(i):
                wa = None
                if i < NA:
                    wa = wpool.tile([128, KT, 512], BF16, tag="wa")
                    dma(out=wa[:, :, :], in_=W4A[i])
                wb = wpool.tile([128, KT, 512], BF16, tag="wb")
                dma(out=wb[:, :, :], in_=W4B[i])
                bt = bpool.tile([1, 1024], BF16, tag="bt")
                dma(out=bt[:, :], in_=b4s[i])
                return wa, wb, bt

            xT_sb = singles.tile([2, B], F32)
            W1_sb = singles.tile([2, 256], F32)
            W2_sb = singles.tile([128, 2, 512], BF16)
            W3_sb = singles.tile([128, 4, 1024], BF16)
            bias_sb = singles.tile([128, 14], F32)
            h1_sb = singles.tile([128, 2, B], BF16)
            h2_sb = singles.tile([128, 4, B], BF16)
            h3_sb = singles.tile([128, KT, B], BF16)
            ones = singles.tile([1, B], BF16)
            eye_sb = singles.tile([128, 64], BF16)
            Ft = singles.tile([128, FR * W], BF16)
            Lf = singles.tile([128, LR * W], BF16)
            Ps_sb = singles.tile([128, HR * W], BF16)

            dma(out=xT_sb[:, :], in_=xT[:, :])
            dma(out=W1_sb[:, :], in_=W1[:, :])
            dma(out=bias_sb[:, :], in_=bias[:, :])
            dma(out=W2_sb[:, :, :], in_=W2[:, :, :])
            dma(out=W3_sb[:, :, :], in_=W3[:, :, :])
            dma(out=eye_sb[:, :], in_=eye[:, :])
            nc.vector.memset(ones, 1.0)

            # ---- MLP (transposed activations: h_T[feat, batch]) ----
            with tc.tile_pool(name="mlp_psum", bufs=2, space="PSUM") as mp:
                for m in range(2):
                    ps = mp.tile([128, B], F32)
                    nc.tensor.matmul(
                        ps, W1_sb[:, m * 128 : (m + 1) * 128], xT_sb[:, :],
                        start=True, stop=True,
                    )
                    nc.scalar.activation(
                        h1_sb[:, m, :], ps, Relu, bias=bias_sb[:, m : m + 1], scale=1.0
                    )
                for m in range(4):
                    ps = mp.tile([128, B], F32)
                    for k in range(2):
                        nc.tensor.matmul(
                            ps, W2_sb[:, k, m * 128 : (m + 1) * 128], h1_sb[:, k, :],
                            start=(k == 0), stop=(k == 1),
                        )
                    nc.scalar.activation(
                        h2_sb[:, m, :], ps, Relu, bias=bias_sb[:, 2 + m : 3 + m], scale=1.0
                    )
                for m in range(8):
                    ps = mp.tile([128, B], F32)
                    for k in range(4):
                        nc.tensor.matmul(
                            ps, W3_sb[:, k, m * 128 : (m + 1) * 128], h2_sb[:, k, :],
                            start=(k == 0), stop=(k == 3),
                        )
                    nc.scalar.activation(
                        h3_sb[:, m, :], ps, Relu, bias=bias_sb[:, 6 + m : 7 + m], scale=1.0
                    )

            # ---- main matmul: F[p, 512-col chunks], half A -> partitions 0-63,
            # half B -> partitions 64-127 (PE column groups run concurrently) ----
            with tc.tile_pool(name="ppool", bufs=3, space="PSUM") as ppool:
                for i in range(PAIRS):
                    wa, wb, bt = fetch_pair(i)
                    has_a = wa is not None
                    # half A accumulates in bank 0 (partitions 0-63), half B in
                    # bank 1 (partitions 64-127): separate psum zero regions,
                    # concurrent PE column groups.
                    ps = ppool.tile([128, 1024], F32)
                    for k in range(KT):
                        if has_a:
                            nc.tensor.matmul(
                                ps[0:64, 0:512], h3_sb[:, k, :], wa[:, k, :],
                                start=(k == 0), stop=False, tile_position=(0, 0),
                            )
                        nc.tensor.matmul(
                            ps[64:128, 512:1024], h3_sb[:, k, :], wb[:, k, :],
                            start=(k == 0), stop=False, tile_position=(0, 64),
                        )
                    if has_a:
                        nc.tensor.matmul(
                            ps[0:64, 0:512], ones[:, :], bt[:, 0:512],
                            start=False, stop=True, tile_position=(0, 0),
                        )
                        nc.scalar.copy(Ft[0:64, i * 512 : (i + 1) * 512], ps[0:64, 0:512])
                    nc.tensor.matmul(
                        ps[64:128, 512:1024], ones[:, :], bt[:, 512:1024],
                        start=False, stop=True, tile_position=(0, 64),
                    )
                    nc.scalar.copy(
                        Ft[64:128, i * 512 : (i + 1) * 512], ps[64:128, 512:1024]
                    )
                    if i == 1:
                        # A-half top rows 16..19 = B-half rows 0..3 (both are
                        # global rows 14..17): SBUF->SBUF partition copy
                        # instead of re-streaming 2.1MB of W4.
                        dma(out=Ft[0:64, 4096:5120], in_=Ft[64:128, 0:1024])

                # ---- stencils, slab-pipelined & interleaved so DVE overlaps
                # with the matmul/DMA stream ----
                Fv = Ft.rearrange("p (r x) -> p r x", x=W)
                Lfv = Lf.rearrange("p (r x) -> p r x", x=W)
                STT = nc.vector.scalar_tensor_tensor

                def lf_slab(j):
                    # Lf rows 3j..3j+2 (center = F row+1)
                    n = 3 * W
                    r0 = 3 * j
                    cb = (r0 + 1) * W
                    s1 = spool.tile([128, n], BF16, tag="s1")
                    s2 = spool.tile([128, n], BF16, tag="s2")
                    nc.vector.tensor_add(s1, Ft[:, cb - 1 : cb - 1 + n], Ft[:, cb + 1 : cb + 1 + n])
                    s1v = s1.rearrange("p (r x) -> p r x", x=W)
                    nc.scalar.mul(s1v[:, :, 0:1], Fv[:, r0 + 1 : r0 + 4, 1:2], 2.0)
                    nc.scalar.mul(s1v[:, :, W - 1 : W], Fv[:, r0 + 1 : r0 + 4, W - 2 : W - 1], 2.0)
                    nc.vector.tensor_add(s2, Ft[:, cb - W : cb - W + n], Ft[:, cb + W : cb + W + n])
                    STT(out=s1, in0=Ft[:, cb : cb + n], scalar=-4.0, in1=s1, op0=MUL, op1=ADD)
                    nc.vector.tensor_add(Lf[:, r0 * W : r0 * W + n], s1, s2)

                def r_slab(m):
                    # residual rows 4m..4m+3 (centers: Lf row+1, F row+2)
                    n = 4 * W
                    r0 = 4 * m
                    lb = (r0 + 1) * W
                    fb = (r0 + 2) * W
                    ob = r0 * W
                    # scalar-engine DMA queue: lands promptly instead of
                    # waiting behind the W4 stream in the sync queue
                    nc.scalar.dma_start(out=Ps_sb[:, ob : ob + n], in_=Ps[:, ob : ob + n])
                    t1 = tpool.tile([128, n], BF16, tag="t1")
                    t2 = tpool.tile([128, n], BF16, tag="t2")
                    t5 = tpool.tile([128, n], BF16, tag="t5")
                    rt = rpool.tile([128, n], F32, tag="rt")
                    nc.vector.tensor_add(t1, Lf[:, lb - 1 : lb - 1 + n], Lf[:, lb + 1 : lb + 1 + n])
                    t1v = t1.rearrange("p (r x) -> p r x", x=W)
                    nc.scalar.mul(t1v[:, :, 0:1], Lfv[:, r0 + 1 : r0 + 5, 1:2], 2.0)
                    nc.scalar.mul(t1v[:, :, W - 1 : W], Lfv[:, r0 + 1 : r0 + 5, W - 2 : W - 1], 2.0)
                    nc.vector.tensor_add(t2, Lf[:, lb - W : lb - W + n], Lf[:, lb + W : lb + W + n])
                    STT(out=t1, in0=Lf[:, lb : lb + n], scalar=-3.0, in1=t1, op0=MUL, op1=ADD)
                    nc.vector.tensor_sub(t5, Ft[:, fb : fb + n], Ps_sb[:, ob : ob + n])
                    nc.vector.tensor_add(t1, t1, t2)
                    nc.vector.tensor_add(rt, t1, t5)
                    dma(out=out[:, ob : ob + n], in_=rt[:, :])

                # R slab m reads Lf rows 4m..4m+5, so it must follow Lf slab
                # ceil((4m+5+1)/3)-1.
                for step in ("L0", "L1", "R0", "L2", "L3", "R1", "L4", "R2", "L5", "R3"):
                    kind, idx = step[0], int(step[1])
                    if kind == "L":
                        lf_slab(idx)
                    else:
                        r_slab(idx)

    nc.compile()
    return nc


def _ext_rows(c):
    """40 mirrored global row indices: 20 for half A, 20 for half B."""
    y0 = c * OWN
    rows_a = [_mirror(y0 - 2 + j) for j in range(FR)]
    rows_b = [_mirror(y0 + HR - 2 + j) for j in range(FR)]
    return rows_a + rows_b


def _prep_shared(inputs):
    f32 = np.float32
    shared = {
        "xT": np.ascontiguousarray(inputs["x_coloc"].T, dtype=f32),
        "W1": np.ascontiguousarray(inputs["W1"], dtype=f32),
        "W2": np.ascontiguousarray(
            np.asarray(inputs["W2"], dtype=f32).reshape(2, 128, 512).transpose(1, 0, 2).astype(BF16_NP)
        ),
        "W3": np.ascontiguousarray(
            np.asarray(inputs["W3"], dtype=f32).reshape(4, 128, 1024).transpose(1, 0, 2).astype(BF16_NP)
        ),
        "bias": np.ascontiguousarray(
            np.concatenate(
                [
                    np.asarray(inputs["b1"], dtype=f32).reshape(2, 128).T,
                    np.asarray(inputs["b2"], dtype=f32).reshape(4, 128).T,
                    np.asarray(inputs["b3"], dtype=f32).reshape(8, 128).T,
                ],
                axis=1,
            )
        ),
        "eye": np.ascontiguousarray(
            np.tile(np.eye(64, dtype=np.float32), (2, 1)).astype(BF16_NP)
        ),
    }
    return shared


def _prep_core(c, W4, b4, P):
    y0 = c * OWN
    # A half: F rows -2..13 streamed (local rows 0..15); local rows 16..19 are
    # copied on-device from the B half. B half: F rows 14..33 fully streamed.
    rows_a = [_mirror(y0 - 2 + j) for j in range(2 * NA)]
    rows_b = [_mirror(y0 + HR - 2 + j) for j in range(FR)]

    W4r = W4.reshape(1024, H, W)

    def chunks(rows):
        G = W4r[:, rows, :].reshape(KT, 128, len(rows) // 2, 512)  # [k,p,chunk,x]
        G = G.transpose(2, 1, 0, 3)                                # [chunk,p,k,x]
        return np.ascontiguousarray(G.astype(BF16_NP))

    W4A_arr = chunks(rows_a)
    W4B_arr = chunks(rows_b)

    gb_a = b4.reshape(H, W)[rows_a].reshape(NA, 512)
    gb_b = b4.reshape(H, W)[rows_b].reshape(PAIRS, 512)
    b4s_arr = np.zeros((PAIRS, 1, 1024), dtype=BF16_NP)
    b4s_arr[:NA, 0, :512] = gb_a.astype(BF16_NP)
    b4s_arr[:, 0, 512:] = gb_b.astype(BF16_NP)

    Pr = P.reshape(B, H, W)
    Ps = np.concatenate(
        [
            Pr[:, y0 : y0 + HR, :].reshape(B, HR * W),
            Pr[:, y0 + HR : y0 + OWN, :].reshape(B, HR * W),
        ],
        axis=0,
    ).astype(BF16_NP)
    return {
        "W4A": W4A_arr, "W4B": W4B_arr, "b4s": b4s_arr,
        "Ps": np.ascontiguousarray(Ps),
    }


def make_in_maps(inputs):
    shared = _prep_shared(inputs)
    W4 = np.asarray(inputs["W4"], dtype=np.float32)
    b4 = np.asarray(inputs["b4"], dtype=np.float32)
    P = np.asarray(inputs["P"], dtype=np.float32)
    in_maps = []
    for c in range(NCORES):
        m = dict(shared)
        m.update(_prep_core(c, W4, b4, P))
        in_maps.append(m)
    return in_maps


def assemble_output(results):
    outf = np.empty((B, H, W), dtype=np.float32)
    for c in range(NCORES):
        oc = np.asarray(results[c]["out"])
        y0 = c * OWN
        outf[:, y0 : y0 + HR, :] = oc[:64].reshape(B, HR, W)
        outf[:, y0 + HR : y0 + OWN, :] = oc[64:].reshape(B, HR, W)
    return outf.reshape(B, H * W)


def get_program():
    if "nc" not in _PROGRAM_CACHE:
        _PROGRAM_CACHE["nc"] = _build_program()
    return _PROGRAM_CACHE["nc"]


def kernel(**inputs):
    nc = get_program()
    in_maps = make_in_maps(inputs)
    res = run_bass_kernel_spmd(nc, in_maps, list(range(NCORES)))
    return assemble_output(res.results)


# revision 53
# speedup vs baseline: 1.0307x; 1.0065x over previous
"""Trainium2 Bass kernel: MechanicsPINN residual (MLP field + biharmonic stencil).

Math (reference): f = MLP(x_coloc) -> [B, H*W]; residual = L(L(f)) + L(f) + f - P
where L is the 5-point reflect-padded Laplacian (EI = KC = GC = 1, dx = dy = 1).

Sharding: tensor-parallel over the 65536 output pixels = 256 image rows.
Core c owns rows [32c, 32c+32). Each core recomputes the tiny MLP, then
computes f for its rows plus a 2-row halo on each side (mirror boundary rows
are folded in on the host by remapping which W4 columns each core streams,
so the device stencil needs no y-boundary cases and no cross-core comms).

Device layout: batch (64) on partitions; each core's 32 rows are split into
two 16-row halves stacked on the partition axis (partitions 0-63 = batch for
half A, 64-127 = batch for half B) via PE column-tiling, so DVE stencil
passes and the big matmul both use all 128 partitions. W4/W2/W3/P are
host-cast to bf16 (halves the HBM traffic that bounds this kernel); stencil
intermediates are bf16 with the final combine accumulated to fp32.
"""

import numpy as np
import ml_dtypes

import concourse.bass as bass
import concourse.tile as tile
from concourse import bacc, mybir
from concourse.bass_utils import run_bass_kernel_spmd

F32 = mybir.dt.float32
BF16 = mybir.dt.bfloat16
BF16_NP = ml_dtypes.bfloat16

B = 64          # batch (collocation samples)
H = 256         # image rows
W = 256         # image cols
NCORES = 8
OWN = 32        # image rows owned per core
HR = 16         # rows per half-slab
FR = 20         # f rows held per half (HR + 2 halo each side)
LR = 18         # laplacian rows per half (HR + 1 each side)
PAIRS = 10      # B-half 512-col chunks (2 rows each), streamed in order
NA = 8          # A-half chunks: rows 0..15; rows 16..19 are copied from the
                # B half (B rows 0..3 are the same global rows 14..17)
KT = 8          # k tiles of the 1024-dim contraction

_PROGRAM_CACHE = {}


def _mirror(j):
    # jnp.pad mode='reflect' (no edge repeat): p[-1] = f[1], p[H] = f[H-2]
    if j < 0:
        return -j
    if j > H - 1:
        return 2 * (H - 1) - j
    return j


def _build_program():
    nc = bacc.Bacc("TRN2", target_bir_lowering=False, debug=False)

    xT = nc.declare_dram_parameter("xT", [2, B], F32, isOutput=False)
    W1 = nc.declare_dram_parameter("W1", [2, 256], F32, isOutput=False)
    W2 = nc.declare_dram_parameter("W2", [128, 2, 512], BF16, isOutput=False)
    W3 = nc.declare_dram_parameter("W3", [128, 4, 1024], BF16, isOutput=False)
    bias = nc.declare_dram_parameter("bias", [128, 14], F32, isOutput=False)
    W4A = nc.declare_dram_parameter("W4A", [NA, 128, KT, 512], BF16, isOutput=False)
    W4B = nc.declare_dram_parameter("W4B", [PAIRS, 128, KT, 512], BF16, isOutput=False)
    b4s = nc.declare_dram_parameter("b4s", [PAIRS, 1, 1024], BF16, isOutput=False)
    Ps = nc.declare_dram_parameter("Ps", [128, HR * W], BF16, isOutput=False)
    out = nc.declare_dram_parameter("out", [128, HR * W], F32, isOutput=True)

    Relu = mybir.ActivationFunctionType.Relu
    MUL = mybir.AluOpType.mult
    ADD = mybir.AluOpType.add

    with tile.TileContext(nc) as tc:
        with (
            tc.tile_pool(name="singles", bufs=1) as singles,
            tc.tile_pool(name="wpool", bufs=5) as wpool,
            tc.tile_pool(name="bpool", bufs=3) as bpool,
            tc.tile_pool(name="spool", bufs=2) as spool,
            tc.tile_pool(name="tpool", bufs=2) as tpool,
            tc.tile_pool(name="rpool", bufs=2) as rpool,
        ):
            dma = nc.sync.dma_start

            def fetch_pair(i):
                wa = None
                if i < NA:
                    wa = wpool.tile([128, KT, 512], BF16, tag="wa")
                    dma(out=wa[:, :, :], in_=W4A[i])
                wb = wpool.tile([128, KT, 512], BF16, tag="wb")
                dma(out=wb[:, :, :], in_=W4B[i])
                bt = bpool.tile([1, 1024], BF16, tag="bt")
                dma(out=bt[:, :], in_=b4s[i])
                return wa, wb, bt

            xT_sb = singles.tile([2, B], F32)
            W1_sb = singles.tile([2, 256], F32)
            W2_sb = singles.tile([128, 2, 512], BF16)
            W3_sb = singles.tile([128, 4, 1024], BF16)
            bias_sb = singles.tile([128, 14], F32)
            h1_sb = singles.tile([128, 2, B], BF16)
            h2_sb = singles.tile([128, 4, B], BF16)
            h3_sb = singles.tile([128, KT, B], BF16)
            ones = singles.tile([1, B], BF16)
            Ft = singles.tile([128, FR * W], BF16)
            Lf = singles.tile([128, LR * W], BF16)
            Ps_sb = singles.tile([128, HR * W], BF16)

            dma(out=xT_sb[:, :], in_=xT[:, :])
            dma(out=W1_sb[:, :], in_=W1[:, :])
            dma(out=bias_sb[:, :], in_=bias[:, :])
            dma(out=W2_sb[:, :, :], in_=W2[:, :, :])
            dma(out=W3_sb[:, :, :], in_=W3[:, :, :])
            nc.vector.memset(ones, 1.0)

            # ---- MLP (transposed activations: h_T[feat, batch]) ----
            with tc.tile_pool(name="mlp_psum", bufs=2, space="PSUM") as mp:
                for m in range(2):
                    ps = mp.tile([128, B], F32)
                    nc.tensor.matmul(
                        ps, W1_sb[:, m * 128 : (m + 1) * 128], xT_sb[:, :],
                        start=True, stop=True,
                    )
                    nc.scalar.activation(
                        h1_sb[:, m, :], ps, Relu, bias=bias_sb[:, m : m + 1], scale=1.0
                    )
                for m in range(4):
                    ps = mp.tile([128, B], F32)
                    for k in range(2):
                        nc.tensor.matmul(
                            ps, W2_sb[:, k, m * 128 : (m + 1) * 128], h1_sb[:, k, :],
                            start=(k == 0), stop=(k == 1),
                        )
                    nc.scalar.activation(
                        h2_sb[:, m, :], ps, Relu, bias=bias_sb[:, 2 + m : 3 + m], scale=1.0
                    )
                for m in range(8):
                    ps = mp.tile([128, B], F32)
                    for k in range(4):
                        nc.tensor.matmul(
                            ps, W3_sb[:, k, m * 128 : (m + 1) * 128], h2_sb[:, k, :],
                            start=(k == 0), stop=(k == 3),
                        )
                    nc.scalar.activation(
                        h3_sb[:, m, :], ps, Relu, bias=bias_sb[:, 6 + m : 7 + m], scale=1.0
                    )

            # ---- main matmul: F[p, 512-col chunks], half A -> partitions 0-63,
            # half B -> partitions 64-127 (PE column groups run concurrently) ----
            with tc.tile_pool(name="ppool", bufs=3, space="PSUM") as ppool:
                for i in range(PAIRS):
                    wa, wb, bt = fetch_pair(i)
                    has_a = wa is not None
                    # half A accumulates in bank 0 (partitions 0-63), half B in
                    # bank 1 (partitions 64-127): separate psum zero regions,
                    # concurrent PE column groups.
                    ps = ppool.tile([128, 1024], F32)
                    for k in range(KT):
                        if has_a:
                            nc.tensor.matmul(
                                ps[0:64, 0:512], h3_sb[:, k, :], wa[:, k, :],
                                start=(k == 0), stop=False, tile_position=(0, 0),
                            )
                        nc.tensor.matmul(
                            ps[64:128, 512:1024], h3_sb[:, k, :], wb[:, k, :],
                            start=(k == 0), stop=False, tile_position=(0, 64),
                        )
                    if has_a:
                        nc.tensor.matmul(
                            ps[0:64, 0:512], ones[:, :], bt[:, 0:512],
                            start=False, stop=True, tile_position=(0, 0),
                        )
                        nc.scalar.copy(Ft[0:64, i * 512 : (i + 1) * 512], ps[0:64, 0:512])
                    nc.tensor.matmul(
                        ps[64:128, 512:1024], ones[:, :], bt[:, 512:1024],
                        start=False, stop=True, tile_position=(0, 64),
                    )
                    nc.scalar.copy(
                        Ft[64:128, i * 512 : (i + 1) * 512], ps[64:128, 512:1024]
                    )
                    if i == 1:
                        # A-half top rows 16..19 = B-half rows 0..3 (both are
                        # global rows 14..17): SBUF->SBUF partition copy
                        # instead of re-streaming 2.1MB of W4.
                        dma(out=Ft[0:64, 4096:5120], in_=Ft[64:128, 0:1024])

                # ---- stencils, slab-pipelined & interleaved so DVE overlaps
                # with the matmul/DMA stream ----
                Fv = Ft.rearrange("p (r x) -> p r x", x=W)
                Lfv = Lf.rearrange("p (r x) -> p r x", x=W)
                STT = nc.vector.scalar_tensor_tensor

                def lf_slab(j):
                    # Lf rows 3j..3j+2 (center = F row+1)
                    n = 3 * W
                    r0 = 3 * j
                    cb = (r0 + 1) * W
                    s1 = spool.tile([128, n], BF16, tag="s1")
                    s2 = spool.tile([128, n], BF16, tag="s2")
                    nc.vector.tensor_add(s1, Ft[:, cb - 1 : cb - 1 + n], Ft[:, cb + 1 : cb + 1 + n])
                    s1v = s1.rearrange("p (r x) -> p r x", x=W)
                    nc.scalar.mul(s1v[:, :, 0:1], Fv[:, r0 + 1 : r0 + 4, 1:2], 2.0)
                    nc.scalar.mul(s1v[:, :, W - 1 : W], Fv[:, r0 + 1 : r0 + 4, W - 2 : W - 1], 2.0)
                    nc.vector.tensor_add(s2, Ft[:, cb - W : cb - W + n], Ft[:, cb + W : cb + W + n])
                    STT(out=s1, in0=Ft[:, cb : cb + n], scalar=-4.0, in1=s1, op0=MUL, op1=ADD)
                    nc.vector.tensor_add(Lf[:, r0 * W : r0 * W + n], s1, s2)

                def r_slab(m):
                    # residual rows 4m..4m+3 (centers: Lf row+1, F row+2)
                    n = 4 * W
                    r0 = 4 * m
                    lb = (r0 + 1) * W
                    fb = (r0 + 2) * W
                    ob = r0 * W
                    # scalar-engine DMA queue: lands promptly instead of
                    # waiting behind the W4 stream in the sync queue
                    nc.scalar.dma_start(out=Ps_sb[:, ob : ob + n], in_=Ps[:, ob : ob + n])
                    t1 = tpool.tile([128, n], BF16, tag="t1")
                    t2 = tpool.tile([128, n], BF16, tag="t2")
                    t5 = tpool.tile([128, n], BF16, tag="t5")
                    rt = rpool.tile([128, n], F32, tag="rt")
                    nc.vector.tensor_add(t1, Lf[:, lb - 1 : lb - 1 + n], Lf[:, lb + 1 : lb + 1 + n])
                    t1v = t1.rearrange("p (r x) -> p r x", x=W)
                    nc.scalar.mul(t1v[:, :, 0:1], Lfv[:, r0 + 1 : r0 + 5, 1:2], 2.0)
                    nc.scalar.mul(t1v[:, :, W - 1 : W], Lfv[:, r0 + 1 : r0 + 5, W - 2 : W - 1], 2.0)
                    nc.vector.tensor_add(t2, Lf[:, lb - W : lb - W + n], Lf[:, lb + W : lb + W + n])
                    STT(out=t1, in0=Lf[:, lb : lb + n], scalar=-3.0, in1=t1, op0=MUL, op1=ADD)
                    nc.vector.tensor_sub(t5, Ft[:, fb : fb + n], Ps_sb[:, ob : ob + n])
                    nc.vector.tensor_add(t1, t1, t2)
                    nc.vector.tensor_add(rt, t1, t5)
                    dma(out=out[:, ob : ob + n], in_=rt[:, :])

                # R slab m reads Lf rows 4m..4m+5, so it must follow Lf slab
                # ceil((4m+5+1)/3)-1.
                for step in ("L0", "L1", "R0", "L2", "L3", "R1", "L4", "R2", "L5", "R3"):
                    kind, idx = step[0], int(step[1])
                    if kind == "L":
                        lf_slab(idx)
                    else:
                        r_slab(idx)

    nc.compile()
    return nc


def _ext_rows(c):
    """40 mirrored global row indices: 20 for half A, 20 for half B."""
    y0 = c * OWN
    rows_a = [_mirror(y0 - 2 + j) for j in range(FR)]
    rows_b = [_mirror(y0 + HR - 2 + j) for j in range(FR)]
    return rows_a + rows_b


def _prep_shared(inputs):
    f32 = np.float32
    shared = {
        "xT": np.ascontiguousarray(inputs["x_coloc"].T, dtype=f32),
        "W1": np.ascontiguousarray(inputs["W1"], dtype=f32),
        "W2": np.ascontiguousarray(
            np.asarray(inputs["W2"], dtype=f32).reshape(2, 128, 512).transpose(1, 0, 2).astype(BF16_NP)
        ),
        "W3": np.ascontiguousarray(
            np.asarray(inputs["W3"], dtype=f32).reshape(4, 128, 1024).transpose(1, 0, 2).astype(BF16_NP)
        ),
        "bias": np.ascontiguousarray(
            np.concatenate(
                [
                    np.asarray(inputs["b1"], dtype=f32).reshape(2, 128).T,
                    np.asarray(inputs["b2"], dtype=f32).reshape(4, 128).T,
                    np.asarray(inputs["b3"], dtype=f32).reshape(8, 128).T,
                ],
                axis=1,
            )
        ),
    }
    return shared


def _prep_core(c, W4, b4, P):
    y0 = c * OWN
    # A half: F rows -2..13 streamed (local rows 0..15); local rows 16..19 are
    # copied on-device from the B half. B half: F rows 14..33 fully streamed.
    rows_a = [_mirror(y0 - 2 + j) for j in range(2 * NA)]
    rows_b = [_mirror(y0 + HR - 2 + j) for j in range(FR)]

    W4r = W4.reshape(1024, H, W)

    def chunks(rows):
        G = W4r[:, rows, :].reshape(KT, 128, len(rows) // 2, 512)  # [k,p,chunk,x]
        G = G.transpose(2, 1, 0, 3)                                # [chunk,p,k,x]
        return np.ascontiguousarray(G.astype(BF16_NP))

    W4A_arr = chunks(rows_a)
    W4B_arr = chunks(rows_b)

    gb_a = b4.reshape(H, W)[rows_a].reshape(NA, 512)
    gb_b = b4.reshape(H, W)[rows_b].reshape(PAIRS, 512)
    b4s_arr = np.zeros((PAIRS, 1, 1024), dtype=BF16_NP)
    b4s_arr[:NA, 0, :512] = gb_a.astype(BF16_NP)
    b4s_arr[:, 0, 512:] = gb_b.astype(BF16_NP)

    Pr = P.reshape(B, H, W)
    Ps = np.concatenate(
        [
            Pr[:, y0 : y0 + HR, :].reshape(B, HR * W),
            Pr[:, y0 + HR : y0 + OWN, :].reshape(B, HR * W),
        ],
        axis=0,
    ).astype(BF16_NP)
    return {
        "W4A": W4A_arr, "W4B": W4B_arr, "b4s": b4s_arr,
        "Ps": np.ascontiguousarray(Ps),
    }


def make_in_maps(inputs):
    shared = _prep_shared(inputs)
    W4 = np.asarray(inputs["W4"], dtype=np.float32)
    b4 = np.asarray(inputs["b4"], dtype=np.float32)
    P = np.asarray(inputs["P"], dtype=np.float32)
    in_maps = []
    for c in range(NCORES):
        m = dict(shared)
        m.update(_prep_core(c, W4, b4, P))
        in_maps.append(m)
    return in_maps


def assemble_output(results):
    outf = np.empty((B, H, W), dtype=np.float32)
    for c in range(NCORES):
        oc = np.asarray(results[c]["out"])
        y0 = c * OWN
        outf[:, y0 : y0 + HR, :] = oc[:64].reshape(B, HR, W)
        outf[:, y0 + HR : y0 + OWN, :] = oc[64:].reshape(B, HR, W)
    return outf.reshape(B, H * W)


def get_program():
    if "nc" not in _PROGRAM_CACHE:
        _PROGRAM_CACHE["nc"] = _build_program()
    return _PROGRAM_CACHE["nc"]


def kernel(**inputs):
    nc = get_program()
    in_maps = make_in_maps(inputs)
    res = run_bass_kernel_spmd(nc, in_maps, list(range(NCORES)))
    return assemble_output(res.results)


# revision 55
# speedup vs baseline: 1.0999x; 1.0671x over previous
"""Trainium2 Bass kernel: MechanicsPINN residual (MLP field + biharmonic stencil).

Math (reference): f = MLP(x_coloc) -> [B, H*W]; residual = L(L(f)) + L(f) + f - P
where L is the 5-point reflect-padded Laplacian (EI = KC = GC = 1, dx = dy = 1).

Sharding: tensor-parallel over the 65536 output pixels = 256 image rows.
Core c owns rows [32c, 32c+32). Each core recomputes the tiny MLP, then
computes f for its rows plus a 2-row halo on each side (mirror boundary rows
are folded in on the host by remapping which W4 columns each core streams,
so the device stencil needs no y-boundary cases and no cross-core comms).

Device layout: batch (64) on partitions; each core's 32 rows are split into
two 16-row halves stacked on the partition axis (partitions 0-63 = batch for
half A, 64-127 = batch for half B) via PE column-tiling, so DVE stencil
passes and the big matmul both use all 128 partitions. W4/W2/W3/P are
host-cast to bf16 (halves the HBM traffic that bounds this kernel); stencil
intermediates are bf16 with the final combine accumulated to fp32.
"""

import numpy as np
import ml_dtypes

import concourse.bass as bass
import concourse.tile as tile
from concourse import bacc, mybir
from concourse.bass_utils import run_bass_kernel_spmd

F32 = mybir.dt.float32
BF16 = mybir.dt.bfloat16
BF16_NP = ml_dtypes.bfloat16

B = 64          # batch (collocation samples)
H = 256         # image rows
W = 256         # image cols
NCORES = 8
OWN = 32        # image rows owned per core
HR = 16         # rows per half-slab
FR = 20         # f rows held per half (HR + 2 halo each side)
LR = 18         # laplacian rows per half (HR + 1 each side)
PAIRS = 10      # B-half 512-col chunks (2 rows each), streamed in order
NA = 8          # A-half chunks: rows 0..15; rows 16..19 are copied from the
                # B half (B rows 0..3 are the same global rows 14..17)
KT = 8          # k tiles of the 1024-dim contraction

_PROGRAM_CACHE = {}


def _mirror(j):
    # jnp.pad mode='reflect' (no edge repeat): p[-1] = f[1], p[H] = f[H-2]
    if j < 0:
        return -j
    if j > H - 1:
        return 2 * (H - 1) - j
    return j


def _build_program():
    nc = bacc.Bacc("TRN2", target_bir_lowering=False, debug=False)

    xT = nc.declare_dram_parameter("xT", [2, B], F32, isOutput=False)
    W1 = nc.declare_dram_parameter("W1", [2, 256], F32, isOutput=False)
    W2 = nc.declare_dram_parameter("W2", [128, 2, 512], BF16, isOutput=False)
    W3 = nc.declare_dram_parameter("W3", [128, 4, 1024], BF16, isOutput=False)
    bias = nc.declare_dram_parameter("bias", [128, 14], F32, isOutput=False)
    W4A = nc.declare_dram_parameter("W4A", [NA, 128, KT, 512], BF16, isOutput=False)
    W4B = nc.declare_dram_parameter("W4B", [PAIRS, 128, KT, 512], BF16, isOutput=False)
    b4s = nc.declare_dram_parameter("b4s", [PAIRS, 1, 1024], BF16, isOutput=False)
    Ps = nc.declare_dram_parameter("Ps", [128, HR * W], BF16, isOutput=False)
    out = nc.declare_dram_parameter("out", [128, HR * W], F32, isOutput=True)

    Relu = mybir.ActivationFunctionType.Relu
    MUL = mybir.AluOpType.mult
    ADD = mybir.AluOpType.add

    with tile.TileContext(nc) as tc:
        with (
            tc.tile_pool(name="singles", bufs=1) as singles,
            tc.tile_pool(name="wpool", bufs=6) as wpool,
            tc.tile_pool(name="bpool", bufs=3) as bpool,
            tc.tile_pool(name="spool", bufs=2) as spool,
            tc.tile_pool(name="tpool", bufs=2) as tpool,
            tc.tile_pool(name="rpool", bufs=2) as rpool,
        ):
            dma = nc.sync.dma_start

            def fetch_pair(i):
                wa = None
                if i < NA:
                    wa = wpool.tile([128, KT, 512], BF16, tag="wa")
                    dma(out=wa[:, :, :], in_=W4A[i])
                wb = wpool.tile([128, KT, 512], BF16, tag="wb")
                dma(out=wb[:, :, :], in_=W4B[i])
                bt = bpool.tile([1, 1024], BF16, tag="bt")
                dma(out=bt[:, :], in_=b4s[i])
                return wa, wb, bt

            xT_sb = singles.tile([2, B], F32)
            W1_sb = singles.tile([2, 256], F32)
            W2_sb = singles.tile([128, 2, 512], BF16)
            W3_sb = singles.tile([128, 4, 1024], BF16)
            bias_sb = singles.tile([128, 14], F32)
            h1_sb = singles.tile([128, 2, B], BF16)
            h2_sb = singles.tile([128, 4, B], BF16)
            h3_sb = singles.tile([128, KT, B], BF16)
            ones = singles.tile([1, B], BF16)
            Ft = singles.tile([128, FR * W], BF16)
            Lf = singles.tile([128, LR * W], BF16)
            Ps_sb = singles.tile([128, HR * W], BF16)

            # MLP weights ride the scalar-engine DMA queue so they land in
            # parallel with the W4 stream on the sync queue
            nc.scalar.dma_start(out=xT_sb[:, :], in_=xT[:, :])
            nc.scalar.dma_start(out=W1_sb[:, :], in_=W1[:, :])
            nc.scalar.dma_start(out=bias_sb[:, :], in_=bias[:, :])
            nc.scalar.dma_start(out=W2_sb[:, :, :], in_=W2[:, :, :])
            nc.scalar.dma_start(out=W3_sb[:, :, :], in_=W3[:, :, :])
            nc.vector.memset(ones, 1.0)

            # ---- MLP (transposed activations: h_T[feat, batch]) ----
            with tc.tile_pool(name="mlp_psum", bufs=2, space="PSUM") as mp:
                for m in range(2):
                    ps = mp.tile([128, B], F32)
                    nc.tensor.matmul(
                        ps, W1_sb[:, m * 128 : (m + 1) * 128], xT_sb[:, :],
                        start=True, stop=True,
                    )
                    nc.scalar.activation(
                        h1_sb[:, m, :], ps, Relu, bias=bias_sb[:, m : m + 1], scale=1.0
                    )
                for m in range(4):
                    ps = mp.tile([128, B], F32)
                    for k in range(2):
                        nc.tensor.matmul(
                            ps, W2_sb[:, k, m * 128 : (m + 1) * 128], h1_sb[:, k, :],
                            start=(k == 0), stop=(k == 1),
                        )
                    nc.scalar.activation(
                        h2_sb[:, m, :], ps, Relu, bias=bias_sb[:, 2 + m : 3 + m], scale=1.0
                    )
                for m in range(8):
                    ps = mp.tile([128, B], F32)
                    for k in range(4):
                        nc.tensor.matmul(
                            ps, W3_sb[:, k, m * 128 : (m + 1) * 128], h2_sb[:, k, :],
                            start=(k == 0), stop=(k == 3),
                        )
                    nc.scalar.activation(
                        h3_sb[:, m, :], ps, Relu, bias=bias_sb[:, 6 + m : 7 + m], scale=1.0
                    )

            # ---- main matmul: F[p, 512-col chunks], half A -> partitions 0-63,
            # half B -> partitions 64-127 (PE column groups run concurrently) ----
            with tc.tile_pool(name="ppool", bufs=3, space="PSUM") as ppool:
                for i in range(PAIRS):
                    wa, wb, bt = fetch_pair(i)
                    has_a = wa is not None
                    # half A accumulates in bank 0 (partitions 0-63), half B in
                    # bank 1 (partitions 64-127): separate psum zero regions,
                    # concurrent PE column groups.
                    ps = ppool.tile([128, 1024], F32)
                    for k in range(KT):
                        if has_a:
                            nc.tensor.matmul(
                                ps[0:64, 0:512], h3_sb[:, k, :], wa[:, k, :],
                                start=(k == 0), stop=False, tile_position=(0, 0),
                            )
                        nc.tensor.matmul(
                            ps[64:128, 512:1024], h3_sb[:, k, :], wb[:, k, :],
                            start=(k == 0), stop=False, tile_position=(0, 64),
                        )
                    if has_a:
                        nc.tensor.matmul(
                            ps[0:64, 0:512], ones[:, :], bt[:, 0:512],
                            start=False, stop=True, tile_position=(0, 0),
                        )
                        nc.scalar.copy(Ft[0:64, i * 512 : (i + 1) * 512], ps[0:64, 0:512])
                    nc.tensor.matmul(
                        ps[64:128, 512:1024], ones[:, :], bt[:, 512:1024],
                        start=False, stop=True, tile_position=(0, 64),
                    )
                    nc.scalar.copy(
                        Ft[64:128, i * 512 : (i + 1) * 512], ps[64:128, 512:1024]
                    )
                    if i == 1:
                        # A-half top rows 16..19 = B-half rows 0..3 (both are
                        # global rows 14..17): SBUF->SBUF partition copy
                        # instead of re-streaming 2.1MB of W4.
                        dma(out=Ft[0:64, 4096:5120], in_=Ft[64:128, 0:1024])

                # ---- stencils, slab-pipelined & interleaved so DVE overlaps
                # with the matmul/DMA stream ----
                Fv = Ft.rearrange("p (r x) -> p r x", x=W)
                Lfv = Lf.rearrange("p (r x) -> p r x", x=W)
                STT = nc.vector.scalar_tensor_tensor

                def lf_slab(j):
                    # Lf rows 3j..3j+2 (center = F row+1)
                    n = 3 * W
                    r0 = 3 * j
                    cb = (r0 + 1) * W
                    s1 = spool.tile([128, n], BF16, tag="s1")
                    s2 = spool.tile([128, n], BF16, tag="s2")
                    nc.vector.tensor_add(s1, Ft[:, cb - 1 : cb - 1 + n], Ft[:, cb + 1 : cb + 1 + n])
                    s1v = s1.rearrange("p (r x) -> p r x", x=W)
                    nc.scalar.mul(s1v[:, :, 0:1], Fv[:, r0 + 1 : r0 + 4, 1:2], 2.0)
                    nc.scalar.mul(s1v[:, :, W - 1 : W], Fv[:, r0 + 1 : r0 + 4, W - 2 : W - 1], 2.0)
                    nc.vector.tensor_add(s2, Ft[:, cb - W : cb - W + n], Ft[:, cb + W : cb + W + n])
                    STT(out=s1, in0=Ft[:, cb : cb + n], scalar=-4.0, in1=s1, op0=MUL, op1=ADD)
                    nc.vector.tensor_add(Lf[:, r0 * W : r0 * W + n], s1, s2)

                def r_slab(m):
                    # residual rows 4m..4m+3 (centers: Lf row+1, F row+2)
                    n = 4 * W
                    r0 = 4 * m
                    lb = (r0 + 1) * W
                    fb = (r0 + 2) * W
                    ob = r0 * W
                    # scalar-engine DMA queue: lands promptly instead of
                    # waiting behind the W4 stream in the sync queue
                    nc.scalar.dma_start(out=Ps_sb[:, ob : ob + n], in_=Ps[:, ob : ob + n])
                    t1 = tpool.tile([128, n], BF16, tag="t1")
                    t2 = tpool.tile([128, n], BF16, tag="t2")
                    t5 = tpool.tile([128, n], BF16, tag="t5")
                    rt = rpool.tile([128, n], F32, tag="rt")
                    nc.vector.tensor_add(t1, Lf[:, lb - 1 : lb - 1 + n], Lf[:, lb + 1 : lb + 1 + n])
                    t1v = t1.rearrange("p (r x) -> p r x", x=W)
                    nc.scalar.mul(t1v[:, :, 0:1], Lfv[:, r0 + 1 : r0 + 5, 1:2], 2.0)
                    nc.scalar.mul(t1v[:, :, W - 1 : W], Lfv[:, r0 + 1 : r0 + 5, W - 2 : W - 1], 2.0)
                    nc.vector.tensor_add(t2, Lf[:, lb - W : lb - W + n], Lf[:, lb + W : lb + W + n])
                    STT(out=t1, in0=Lf[:, lb : lb + n], scalar=-3.0, in1=t1, op0=MUL, op1=ADD)
                    nc.vector.tensor_sub(t5, Ft[:, fb : fb + n], Ps_sb[:, ob : ob + n])
                    nc.vector.tensor_add(t1, t1, t2)
                    nc.vector.tensor_add(rt, t1, t5)
                    dma(out=out[:, ob : ob + n], in_=rt[:, :])

                # R slab m reads Lf rows 4m..4m+5, so it must follow Lf slab
                # ceil((4m+5+1)/3)-1.
                for step in ("L0", "L1", "R0", "L2", "L3", "R1", "L4", "R2", "L5", "R3"):
                    kind, idx = step[0], int(step[1])
                    if kind == "L":
                        lf_slab(idx)
                    else:
                        r_slab(idx)

    nc.compile()
    return nc


def _ext_rows(c):
    """40 mirrored global row indices: 20 for half A, 20 for half B."""
    y0 = c * OWN
    rows_a = [_mirror(y0 - 2 + j) for j in range(FR)]
    rows_b = [_mirror(y0 + HR - 2 + j) for j in range(FR)]
    return rows_a + rows_b


def _prep_shared(inputs):
    f32 = np.float32
    shared = {
        "xT": np.ascontiguousarray(inputs["x_coloc"].T, dtype=f32),
        "W1": np.ascontiguousarray(inputs["W1"], dtype=f32),
        "W2": np.ascontiguousarray(
            np.asarray(inputs["W2"], dtype=f32).reshape(2, 128, 512).transpose(1, 0, 2).astype(BF16_NP)
        ),
        "W3": np.ascontiguousarray(
            np.asarray(inputs["W3"], dtype=f32).reshape(4, 128, 1024).transpose(1, 0, 2).astype(BF16_NP)
        ),
        "bias": np.ascontiguousarray(
            np.concatenate(
                [
                    np.asarray(inputs["b1"], dtype=f32).reshape(2, 128).T,
                    np.asarray(inputs["b2"], dtype=f32).reshape(4, 128).T,
                    np.asarray(inputs["b3"], dtype=f32).reshape(8, 128).T,
                ],
                axis=1,
            )
        ),
    }
    return shared


def _prep_core(c, W4, b4, P):
    y0 = c * OWN
    # A half: F rows -2..13 streamed (local rows 0..15); local rows 16..19 are
    # copied on-device from the B half. B half: F rows 14..33 fully streamed.
    rows_a = [_mirror(y0 - 2 + j) for j in range(2 * NA)]
    rows_b = [_mirror(y0 + HR - 2 + j) for j in range(FR)]

    W4r = W4.reshape(1024, H, W)

    def chunks(rows):
        G = W4r[:, rows, :].reshape(KT, 128, len(rows) // 2, 512)  # [k,p,chunk,x]
        G = G.transpose(2, 1, 0, 3)                                # [chunk,p,k,x]
        return np.ascontiguousarray(G.astype(BF16_NP))

    W4A_arr = chunks(rows_a)
    W4B_arr = chunks(rows_b)

    gb_a = b4.reshape(H, W)[rows_a].reshape(NA, 512)
    gb_b = b4.reshape(H, W)[rows_b].reshape(PAIRS, 512)
    b4s_arr = np.zeros((PAIRS, 1, 1024), dtype=BF16_NP)
    b4s_arr[:NA, 0, :512] = gb_a.astype(BF16_NP)
    b4s_arr[:, 0, 512:] = gb_b.astype(BF16_NP)

    Pr = P.reshape(B, H, W)
    Ps = np.concatenate(
        [
            Pr[:, y0 : y0 + HR, :].reshape(B, HR * W),
            Pr[:, y0 + HR : y0 + OWN, :].reshape(B, HR * W),
        ],
        axis=0,
    ).astype(BF16_NP)
    return {
        "W4A": W4A_arr, "W4B": W4B_arr, "b4s": b4s_arr,
        "Ps": np.ascontiguousarray(Ps),
    }


def make_in_maps(inputs):
    shared = _prep_shared(inputs)
    W4 = np.asarray(inputs["W4"], dtype=np.float32)
    b4 = np.asarray(inputs["b4"], dtype=np.float32)
    P = np.asarray(inputs["P"], dtype=np.float32)
    in_maps = []
    for c in range(NCORES):
        m = dict(shared)
        m.update(_prep_core(c, W4, b4, P))
        in_maps.append(m)
    return in_maps


def assemble_output(results):
    outf = np.empty((B, H, W), dtype=np.float32)
    for c in range(NCORES):
        oc = np.asarray(results[c]["out"])
        y0 = c * OWN
        outf[:, y0 : y0 + HR, :] = oc[:64].reshape(B, HR, W)
        outf[:, y0 + HR : y0 + OWN, :] = oc[64:].reshape(B, HR, W)
    return outf.reshape(B, H * W)


def get_program():
    if "nc" not in _PROGRAM_CACHE:
        _PROGRAM_CACHE["nc"] = _build_program()
    return _PROGRAM_CACHE["nc"]


def kernel(**inputs):
    nc = get_program()
    in_maps = make_in_maps(inputs)
    res = run_bass_kernel_spmd(nc, in_maps, list(range(NCORES)))
    return assemble_output(res.results)
